# revision 1
# baseline (speedup 1.0000x reference)
"""Trainium2 Bass kernel for nn_BinaryBiaffine2 (biaffine dependency scorer).

Math (per batch b):
    h_dep  = leaky_relu(hidden @ W_dep  + b_dep)             [L, 500]
    h_head = leaky_relu(hidden @ W_head + b_head)            [L, 500]
    dep    = h_dep  @ Wc[:500]                               [L, 2]
    head   = h_head @ Wc[500:]                               [L, 2]
    out[i, j, c] = dep[i, c] + head[j, c] + bc[c]            [L, L, 2]

Sharding: data-parallel over batch, 2 batches per core on 8 cores.

Per-core strategy (v4):
  - hidden loaded natural ([tok, d]) as f32r, PE-transposed 128x128 into
    hT tiles [d, tok] stored bf16; weights bf16 (half the HBM traffic,
    full-rate matmuls at any moving size).
  - MLP in [m, tok] layout; leaky(x+b) = relu(0.99x+0.99b) + 0.01*(x+b),
    linear term bf16 so the DVE add runs in 2x mode.
  - head scores broadcast across partitions by a matmul with a
    partition-replicated Wc column as stationary (+bc folded in the copy).
  - dep scores computed directly in [tok, 2] layout per 128-token i-tile
    (lh chunk stationary, Wc moving) -- a 2-row matmul, nearly free, with
    no transpose chain.
  - dep branch runs in 256-token quarters; each quarter's scores are
    emitted one mt-block into the NEXT quarter's matmuls so the PE never
    stalls but stores start as early as possible.  All DMAs issue from
    the SP queue in readiness order (the hw has one shared DMA device).
  - batch 1 computes its head branch in halves: quarters 0-1 store the
    j<512 half-rows while head-h1 is still being computed, then backfill;
    quarters 2-3 store full rows.  The store stream is sized to keep the
    DMA device saturated from the moment input loads finish until the
    final i-tile.
"""

import os
import sys

if "/opt/trn_rl_repo" not in sys.path:
    sys.path.insert(0, "/opt/trn_rl_repo")

import numpy as np

B, L, D = 16, 1024, 1024
MLP = 500
MLP_PAD = 512
NEG_SLOPE = 0.01
N_CORES = 8
B_PER_CORE = B // N_CORES
P = 128
N_MT = MLP_PAD // P  # 4 m-tiles of 128
N_KO = D // P        # 8 d-slices of 128
N_TSUB = L // P      # 8 token subtiles per batch
NQ = 4               # dep-branch quarters per batch (256 tok each)
QTOK = L // NQ

_CACHE = {}


def _build_nc():
    import concourse.tile as tile
    from concourse import bacc, mybir
    from concourse.bass import ts
    from contextlib import ExitStack

    f32 = mybir.dt.float32
    f32r = mybir.dt.float32r
    bf16 = mybir.dt.bfloat16
    Relu = mybir.ActivationFunctionType.Relu
    Identity = mybir.ActivationFunctionType.Identity
    Add = mybir.AluOpType.add
    Mult = mybir.AluOpType.mult

    nc = bacc.Bacc()

    hidden = nc.dram_tensor("hidden", [B_PER_CORE, L, D], f32r, kind="ExternalInput")
    w_dep_d = nc.dram_tensor("w_dep", [D, MLP_PAD], bf16, kind="ExternalInput")
    w_head_d = nc.dram_tensor("w_head", [D, MLP_PAD], bf16, kind="ExternalInput")
    # bias tiles: columns (2*mt, 2*mt+1) = (0.99*b, b) for m-tile mt
    b_dep_d = nc.dram_tensor("b_dep_t", [P, 2 * N_MT], f32, kind="ExternalInput")
    b_head_d = nc.dram_tensor("b_head_t", [P, 2 * N_MT], f32, kind="ExternalInput")
    wc_dep_d = nc.dram_tensor("wc_dep_t", [P, N_MT, 2], bf16, kind="ExternalInput")
    wc_head_d = nc.dram_tensor("wc_head_bc", [P, 2, N_MT, P], bf16, kind="ExternalInput")
    bc_d = nc.dram_tensor("bc_bc", [P, 2], f32, kind="ExternalInput")
    ident_d = nc.dram_tensor("ident", [P, P], f32r, kind="ExternalInput")
    out_d = nc.dram_tensor("out", [B_PER_CORE, L, L, 2], f32, kind="ExternalOutput")

    with tile.TileContext(nc) as tc:
        with ExitStack() as ctx:
            const = ctx.enter_context(tc.tile_pool(name="const", bufs=1))
            hnat_p = ctx.enter_context(tc.tile_pool(name="hnat", bufs=3))
            hT_p = ctx.enter_context(tc.tile_pool(name="hT", bufs=20))
            lh_p = ctx.enter_context(tc.tile_pool(name="lh", bufs=5))
            lhd_p = ctx.enter_context(tc.tile_pool(name="lhd", bufs=9))
            tmp_p = ctx.enter_context(tc.tile_pool(name="tmp", bufs=3))
            depsc_p = ctx.enter_context(tc.tile_pool(name="depsc", bufs=2))
            hbc_p = ctx.enter_context(tc.tile_pool(name="hbc", bufs=4))
            out_p = ctx.enter_context(tc.tile_pool(name="outp", bufs=4))
            outh_p = ctx.enter_context(tc.tile_pool(name="outh", bufs=5))
            tr_ps = ctx.enter_context(tc.tile_pool(name="trps", bufs=4, space="PSUM"))
            h_ps = ctx.enter_context(tc.tile_pool(name="hps", bufs=4, space="PSUM"))

            # ---- DMA issue order (single SP queue = device order):
            # ident, b0 tokens 0-511, w_head(+b_head), b0 tokens 512-1023,
            # w_dep, small consts, b1 tokens, then stores in readiness order.
            ident_sb = const.tile([P, P], f32r)
            nc.sync.dma_start(ident_sb, ident_d[:, :])

            hid_r = hidden[:, :, :]

            def load_batch(b):
                h_nats = []
                for tp in range(N_TSUB // 2):
                    h_nat = hnat_p.tile([P, 2, D], f32r, name="h_nat")
                    nc.sync.dma_start(
                        h_nat,
                        hid_r[b, ts(tp, 2 * P), :].rearrange(
                            "(s p) d -> p s d", p=P
                        ),
                    )
                    h_nats.append(h_nat)
                return h_nats

            w_sb = {}
            b_sb = {}
            loaded = []

            def load_b0_half(half):
                for tp in range(2 * half, 2 * half + 2):
                    h_nat = hnat_p.tile([P, 2, D], f32r, name="h_nat")
                    for s in range(2):
                        nc.sync.dma_start(
                            h_nat[:, s], hid_r[0, ts(2 * tp + s, P), :]
                        )
                    loaded.append(h_nat)

            load_b0_half(0)
            w_head_sb = const.tile([P, N_KO, MLP_PAD], bf16)
            nc.sync.dma_start(
                w_head_sb, w_head_d[:, :].rearrange("(ko p) m -> p ko m", p=P)
            )
            b_head_sb = const.tile([P, 2 * N_MT], f32)
            nc.sync.dma_start(b_head_sb, b_head_d[:, :])
            load_b0_half(1)
            w_dep_sb = const.tile([P, N_KO, MLP_PAD], bf16)
            nc.sync.dma_start(
                w_dep_sb, w_dep_d[:, :].rearrange("(ko p) m -> p ko m", p=P)
            )
            w_sb["dep"], w_sb["head"] = w_dep_sb, w_head_sb
            b_dep_sb = const.tile([P, 2 * N_MT], f32)
            nc.sync.dma_start(b_dep_sb, b_dep_d[:, :])
            b_sb["dep"], b_sb["head"] = b_dep_sb, b_head_sb
            wc_dep_sb = const.tile([P, N_MT, 2], bf16)
            nc.sync.dma_start(wc_dep_sb, wc_dep_d[:, :, :])
            bc_sb = const.tile([P, 2], f32)
            nc.sync.dma_start(bc_sb, bc_d[:, :])
            wc_head_sb = const.tile([P, 2, N_MT, P], bf16)
            nc.sync.dma_start(wc_head_sb, wc_head_d[:, :, :, :])

            def eng_ring(seq):
                i = [0]

                def nxt():
                    e = seq[i[0] % len(seq)]
                    i[0] += 1
                    return e

                return nxt

            def transposes(h_nats, halves=(0, 1), hTs=None):
                """PE-transpose a batch into hT tiles [d=128, tok=512] bf16."""
                if hTs is None:
                    hTs = {}
                cp = eng_ring([nc.vector, nc.vector, nc.scalar])
                for half in halves:
                    for ko in range(N_KO):
                        ptr = tr_ps.tile([P, 512], f32r, name="ptr")
                        for q in range(4):
                            tsub = half * 4 + q
                            nc.tensor.matmul(
                                ptr[:, ts(q, P)],
                                lhsT=h_nats[tsub // 2][:, tsub % 2, ts(ko, P)],
                                rhs=ident_sb,
                                is_transpose=True,
                                start=True,
                                stop=True,
                            )
                        hT = hT_p.tile([P, 512], bf16, name="hT")
                        e = cp()
                        if e is nc.scalar:
                            e.activation(hT, ptr, Identity)
                        else:
                            e.tensor_copy(hT, ptr)
                        hTs[half, ko] = hT
                return hTs

            def leaky(dst, ps, br, mt):
                """dst(bf16) = leaky_relu(ps + b) via relu(0.99x+0.99b) +
                0.01(x+b); the bf16 add runs in DVE 2x mode."""
                lt = tmp_p.tile(list(dst.shape), bf16, name="lt")
                nc.scalar.activation(
                    dst, ps, Relu,
                    bias=b_sb[br][:, 2 * mt : 2 * mt + 1],
                    scale=1.0 - NEG_SLOPE,
                )
                nc.vector.tensor_scalar(
                    lt, ps,
                    b_sb[br][:, 2 * mt + 1 : 2 * mt + 2], NEG_SLOPE,
                    Add, Mult,
                )
                nc.vector.tensor_add(dst, dst, lt)

            def head_mt_cols(hTs, jq, mt, tiles):
                """Head MLP m-tile for one 256-col j-quarter."""
                half, qc = jq // 2, jq % 2
                ps = h_ps.tile([P, QTOK], f32, name="hps",
                               padded_shape=[P, 512])
                for ko in range(N_KO):
                    nc.tensor.matmul(
                        ps,
                        lhsT=w_sb["head"][:, ko, ts(mt, P)],
                        rhs=hTs[half, ko][:, ts(qc, QTOK)],
                        start=(ko == 0),
                        stop=(ko == N_KO - 1),
                    )
                leaky(tiles[mt][:, ts(jq, QTOK)], ps, "head", mt)

            def bc_cols(lh_tiles, jq, hb_tiles):
                for c in range(2):
                    pbc = h_ps.tile([P, QTOK], f32, name="hps",
                                    padded_shape=[P, 512])
                    for mt in range(N_MT):
                        nc.tensor.matmul(
                            pbc,
                            lhsT=wc_head_sb[:, c, mt, :],
                            rhs=lh_tiles[mt][:, ts(jq, QTOK)],
                            start=(mt == 0),
                            stop=(mt == N_MT - 1),
                        )
                    nc.scalar.activation(
                        hb_tiles[c][:, ts(jq, QTOK)],
                        pbc,
                        Identity,
                        bias=bc_sb[:, c : c + 1],
                    )

            def head_mt(hTs, half, mt, tiles):
                ps = h_ps.tile([P, 512], f32, name="hps")
                for ko in range(N_KO):
                    nc.tensor.matmul(
                        ps,
                        lhsT=w_sb["head"][:, ko, ts(mt, P)],
                        rhs=hTs[half, ko],
                        start=(ko == 0),
                        stop=(ko == N_KO - 1),
                    )
                leaky(tiles[mt][:, ts(half, 512)], ps, "head", mt)

            def head_mlp(hTs, halves, tiles=None):
                if tiles is None:
                    tiles = {mt: lh_p.tile([P, L], bf16, name="lh")
                             for mt in range(N_MT)}
                for half in halves:
                    for mt in range(N_MT):
                        head_mt(hTs, half, mt, tiles)
                return tiles

            def head_bc_phase(lh_tiles, halves, hb_tiles):
                """Head scores, partition-broadcast, +bc folded; fills the
                [:, half*512:...] columns of hb_tiles[c]."""
                for half in halves:
                    for c in range(2):
                        pbc = h_ps.tile([P, 512], f32, name="hps")
                        for mt in range(N_MT):
                            nc.tensor.matmul(
                                pbc,
                                lhsT=wc_head_sb[:, c, mt, :],
                                rhs=lh_tiles[mt][:, ts(half, 512)],
                                start=(mt == 0),
                                stop=(mt == N_MT - 1),
                            )
                        nc.scalar.activation(
                            hb_tiles[c][:, ts(half, 512)],
                            pbc,
                            Identity,
                            bias=bc_sb[:, c : c + 1],
                        )

            def dep_mt(hTs, q, mt):
                """One m-tile of the dep MLP for token quarter q."""
                half, qc = q // 2, q % 2
                psd = h_ps.tile([P, QTOK], f32, name="hps",
                                padded_shape=[P, 512])
                for ko in range(N_KO):
                    nc.tensor.matmul(
                        psd,
                        lhsT=w_sb["dep"][:, ko, ts(mt, P)],
                        rhs=hTs[half, ko][:, ts(qc, QTOK)],
                        start=(ko == 0),
                        stop=(ko == N_KO - 1),
                    )
                lh = lhd_p.tile([P, QTOK], bf16, name="lhd")
                leaky(lh, psd, "dep", mt)
                return lh

            def dep_quarter_mm(hTs, q, mts=tuple(range(N_MT)), lhq=None):
                if lhq is None:
                    lhq = {}
                for mt in mts:
                    lhq[mt] = dep_mt(hTs, q, mt)
                return lhq

            def dep_scores_tile(lhq, q, t, dep_all):
                """Scores for i-tile t of quarter q, directly in [tok, 2]
                layout (lh chunk stationary, wc moving)."""
                psq = tr_ps.tile([P, 2], f32, name="ptr",
                                 padded_shape=[P, 512])
                for mt in range(N_MT):
                    nc.tensor.matmul(
                        psq,
                        lhsT=lhq[mt][:, ts(t, P)],
                        rhs=wc_dep_sb[:, mt, :],
                        start=(mt == 0),
                        stop=(mt == N_MT - 1),
                    )
                col = 4 * q + 2 * t
                nc.vector.tensor_copy(dep_all[:, col : col + 2], psq)

            def asm_tile(b, q, t, dep_all, hb_tiles, eng, jhalves):
                """Assemble + store out rows for i-tile t of quarter q.
                jhalves None = one full-row [P, L, 2] tile."""
                tsub = 2 * q + t
                d0 = dep_all[:, 4 * q + 2 * t : 4 * q + 2 * t + 1]
                d1 = dep_all[:, 4 * q + 2 * t + 1 : 4 * q + 2 * t + 2]

                def emit(ot, c, dap, srcv):
                    e = eng()
                    if e is nc.scalar:
                        e.activation(ot, srcv, Identity, bias=dap)
                    else:
                        e.tensor_scalar(ot, srcv, dap, None, Add)

                if jhalves is None:
                    ot = out_p.tile([P, L, 2], f32, name="ot")
                    for c, dap in ((0, d0), (1, d1)):
                        emit(ot[:, :, c], c, dap, hb_tiles[c])
                    nc.sync.dma_start(out_d[b, ts(tsub, P)], ot)
                else:
                    for jh in jhalves:
                        ot = outh_p.tile([P, 512, 2], f32, name="oth")
                        for c, dap in ((0, d0), (1, d1)):
                            emit(ot[:, :, c], c, dap,
                                 hb_tiles[c][:, ts(jh, 512)])
                        nc.sync.dma_start(
                            out_d[b, ts(tsub, P), ts(jh, 512)], ot
                        )

            def asm_cols(b, q, t, dep_all, hb_tiles, eng, j0, w):
                """Assemble + store out[b, i-tile, j0:j0+w, :]."""
                tsub = 2 * q + t
                d0 = dep_all[:, 4 * q + 2 * t : 4 * q + 2 * t + 1]
                d1 = dep_all[:, 4 * q + 2 * t + 1 : 4 * q + 2 * t + 2]
                pool = outh_p if w <= 512 else out_p
                ot = pool.tile([P, w, 2], f32, name="oc")
                for c, dap in ((0, d0), (1, d1)):
                    e = eng()
                    srcv = hb_tiles[c][:, j0 : j0 + w]
                    if e is nc.scalar:
                        e.activation(ot[:, :, c], srcv, Identity, bias=dap)
                    else:
                        e.tensor_scalar(ot[:, :, c], srcv, dap, None, Add)
                nc.sync.dma_start(
                    out_d[b, ts(tsub, P), j0 : j0 + w], ot
                )

            def sq_asm(b, q, lhq, dep_all, hb, engs, jhalves=None):
                eng = eng_ring(engs)
                for t in range(2):
                    dep_scores_tile(lhq, q, t, dep_all)
                    asm_tile(b, q, t, dep_all, hb, eng, jhalves)

            # ================= batch 0 =================
            # PE: T0h0, H0h0, T0h1, H0h1, bc -- head (and with it the first
            # stores) starts as soon as half-0 tokens and w_head land.
            hTs0 = {}
            transposes(loaded, (0,), hTs0)
            lh_head = head_mlp(hTs0, (0,))
            transposes(loaded, (1,), hTs0)
            b1_nats = load_batch(1)
            head_mlp(hTs0, (1,), tiles=lh_head)
            hb0 = {c: hbc_p.tile([P, L], f32, name="hb") for c in range(2)}
            head_bc_phase(lh_head, (0, 1), hb0)

            # dep quarters; scores for quarter q are emitted one mt-block
            # into quarter q+1 so leaky latency is hidden but stores issue
            # at the earliest possible moment.  b1's transposes slot into
            # the stream where they do not delay stores.
            dep_all0 = depsc_p.tile([P, 4 * NQ], f32, name="dep_all")
            hTs1 = {}
            A, V, G = nc.scalar, nc.vector, nc.gpsimd
            lhq0 = dep_quarter_mm(hTs0, 0)
            lhq1 = dep_quarter_mm(hTs0, 1, (0,))
            sq_asm(0, 0, lhq0, dep_all0, hb0, [A, V, A, V])
            dep_quarter_mm(hTs0, 1, (1, 2, 3), lhq1)
            transposes(b1_nats, (0,), hTs1)
            lhq2 = dep_quarter_mm(hTs0, 2, (0,))
            sq_asm(0, 1, lhq1, dep_all0, hb0, [G, A, V, G])
            dep_quarter_mm(hTs0, 2, (1, 2, 3), lhq2)
            transposes(b1_nats, (1,), hTs1)
            lhq3 = dep_quarter_mm(hTs0, 3, (0,))
            sq_asm(0, 2, lhq2, dep_all0, hb0, [G, V, A, G])
            dep_quarter_mm(hTs0, 3, (1, 2, 3), lhq3)
            sq_asm(0, 3, lhq3, dep_all0, hb0, [G, A, V, G])

            # ================= batch 1 (last) =================
            lh_head = head_mlp(hTs1, (0,))
            hb1 = {c: hbc_p.tile([P, L], f32, name="hb") for c in range(2)}
            head_bc_phase(lh_head, (0,), hb1)

            dep_all1 = depsc_p.tile([P, 4 * NQ], f32, name="dep_all")
            # quarters 0-1: j<512 halves store while head-h1 runs
            lhq0 = dep_quarter_mm(hTs1, 0)
            lhq1 = dep_quarter_mm(hTs1, 1, (0,))
            sq_asm(1, 0, lhq0, dep_all1, hb1, [A, V, A, V], (0,))
            dep_quarter_mm(hTs1, 1, (1, 2, 3), lhq1)
            head_mt_cols(hTs1, 2, 0, lh_head)
            sq_asm(1, 1, lhq1, dep_all1, hb1, [A, V, A, V], (0,))
            # head j-quarter 2, then backfill j 512:768 for i-quarters 0-1
            for mt in (1, 2, 3):
                head_mt_cols(hTs1, 2, mt, lh_head)
            bc_cols(lh_head, 2, hb1)
            # dep i-quarter 2 first so its leaky ops are not queued behind
            # the backfill assemblies on ACT/DVE; it stores j 0:768 in one go
            lhq2 = dep_quarter_mm(hTs1, 2)
            e_bf2 = eng_ring([G, V, G, A, G, V, A, G])
            for q in (0, 1):
                for t in range(2):
                    asm_cols(1, q, t, dep_all1, hb1, e_bf2, 512, QTOK)
            head_mt_cols(hTs1, 3, 0, lh_head)
            eng = eng_ring([A, V, A, V])
            for t in range(2):
                dep_scores_tile(lhq2, 2, t, dep_all1)
                asm_cols(1, 2, t, dep_all1, hb1, eng, 0, 768)
            # head j-quarter 3, backfill j 768: for i-quarters 0-2
            for mt in (1, 2, 3):
                head_mt_cols(hTs1, 3, mt, lh_head)
            bc_cols(lh_head, 3, hb1)
            # dep i-quarter 3 mm/leaky first, backfills after
            lhq3 = dep_quarter_mm(hTs1, 3)
            e_bf3 = eng_ring([G, V, G, A, G, V, A, G, G, V, A, G])
            for q in (0, 1, 2):
                for t in range(2):
                    asm_cols(1, q, t, dep_all1, hb1, e_bf3, 768, QTOK)
            eng = eng_ring([A, V, V, A])
            for t in range(2):
                dep_scores_tile(lhq3, 3, t, dep_all1)
                asm_cols(1, 3, t, dep_all1, hb1, eng, 0, L)

    nc.compile()
    return nc


def _prep_consts(W_dep, b_dep, W_head, b_head, Wc, bc):
    import ml_dtypes

    f = np.float32
    bf = ml_dtypes.bfloat16

    def pad_w(W):
        Wp = np.zeros((D, MLP_PAD), f)
        Wp[:, :MLP] = W
        return Wp.astype(bf)

    def bias_t(bvec):
        bp = np.zeros((MLP_PAD,), f)
        bp[:MLP] = bvec
        bt = bp.reshape(N_MT, P).T  # [P, N_MT]
        out = np.empty((P, 2 * N_MT), f)
        out[:, 0::2] = (1.0 - NEG_SLOPE) * bt
        out[:, 1::2] = bt
        return out

    wc_dep_pad = np.zeros((MLP_PAD, 2), f)
    wc_dep_pad[:MLP] = Wc[:MLP]
    wc_dep_t = wc_dep_pad.reshape(N_MT, P, 2).transpose(1, 0, 2).copy()  # [P,mt,2]

    wc_head_pad = np.zeros((MLP_PAD, 2), f)
    wc_head_pad[:MLP] = Wc[MLP:]
    wh = wc_head_pad.reshape(N_MT, P, 2).transpose(1, 2, 0)  # [P, 2, N_MT]
    wc_head_bc = np.broadcast_to(wh[:, :, :, None], (P, 2, N_MT, P)).copy()

    return {
        "w_dep": pad_w(W_dep),
        "w_head": pad_w(W_head),
        "b_dep_t": bias_t(b_dep),
        "b_head_t": bias_t(b_head),
        "wc_dep_t": wc_dep_t.astype(bf),
        "wc_head_bc": wc_head_bc.astype(bf),
        "bc_bc": np.broadcast_to(bc.astype(f), (P, 2)).copy(),
        "ident": np.eye(P, dtype=f),
    }


def kernel(hidden_state, W_dep, b_dep, W_head, b_head, Wc, bc):
    from concourse.bass_utils import run_bass_kernel_spmd

    hidden_state = np.ascontiguousarray(np.asarray(hidden_state, dtype=np.float32))
    consts = _prep_consts(
        np.asarray(W_dep, np.float32),
        np.asarray(b_dep, np.float32),
        np.asarray(W_head, np.float32),
        np.asarray(b_head, np.float32),
        np.asarray(Wc, np.float32),
        np.asarray(bc, np.float32),
    )

    if "nc" not in _CACHE:
        _CACHE["nc"] = _build_nc()
    nc = _CACHE["nc"]

    in_maps = []
    for k in range(N_CORES):
        m = {"hidden": hidden_state[k * B_PER_CORE : (k + 1) * B_PER_CORE]}
        m.update(consts)
        in_maps.append(m)

    trace = bool(int(os.environ.get("BB_TRACE", "0")))
    if not trace:
        # The NTFF profiling hook (antenv.axon_hooks) is absent in this
        # container; a stray BASS_TRACE=1 would crash the run. Force off.
        os.environ["BASS_NEVER_TRACE"] = "1"
    res = run_bass_kernel_spmd(nc, in_maps, list(range(N_CORES)), trace=trace)
    _CACHE["last_results"] = res
    out = np.concatenate([res.results[k]["out"] for k in range(N_CORES)], axis=0)
    return out

